# revision 30
# baseline (speedup 1.0000x reference)
"""Trainium2 Bass kernel for BPTAttentionWrapperWithAlibi.

Math (validated against reference):
  fused = hs @ W_qkv.T + b_qkv -> q,k,v  [b,s,nh,hd]
  pinv(q_bnsd) == (q^T q)^-1 q^T  (normal equations; cond(G) ~ 5)
  offset_k = inv_norm*k + alibi * (G^-1 q)
  per-position cross-head softmax (32x32), no max-subtraction needed
  (|logits| < 1), ctx @ W_dense.T + b_dense + residual

Sharding: 8 cores = (batch b = c//4) x (512-token slice). The only
cross-core coupling is the Gram matrix G = q^T q per (b,head), handled
by a 1MB AllReduce over each batch's 4 cores, overlapped with the k/v
part of the QKV GEMM. G^-1 via Newton-Schulz on-device, interleaved
into the k/v GEMM stream (two parallel 4-head block chains, one
software-pipeline stage apart so the PE never waits on the DVE).
Small gather DMAs (Gram sum, alibi rows) issue from the ScalarE HWDGE
queue so they are not head-of-line blocked behind the weight stream on
the Sync queue. All matmuls bf16/fp8 (fp32 PSUM accumulation).
"""

import os
import sys

sys.path.insert(0, "/opt/trn_rl_repo")

from contextlib import ExitStack

import ml_dtypes
import numpy as np

import concourse.bass as bass
import concourse.bass_isa as bass_isa
import concourse.mybir as mybir
import concourse.tile as tile
from concourse import bacc
from concourse.bass_utils import run_bass_kernel_spmd

bf16 = ml_dtypes.bfloat16
f8 = ml_dtypes.float8_e4m3fn
F32 = mybir.dt.float32
BF16 = mybir.dt.bfloat16
F8 = mybir.dt.float8e4
DR = mybir.MatmulPerfMode.DoubleRow
WSCALE = 64.0        # fp8 weight pre-scale (W rms ~0.016 is subnormal in e4m3)

B, S, H, NH, HD = 2, 2048, 4096, 32, 128
TOK = 512            # tokens per core
N_CORES = 8
NKT = H // 128       # 32 k-tiles over hidden dim
NCT = 3 * H // 128   # 96 col-tiles over qkv dim
NOC = H // 128       # 32 out-col tiles for dense
SCALE = float(HD) ** -0.5
C1 = 1.0 / 128.0     # inv_norm_factor * scale
EPS = 1e-8
NS_ITERS = int(os.environ.get("NS_ITERS", "3"))
NGRP = TOK // 4      # 128 groups of 4 positions

_CACHE = {}


def _consts():
    ident_bf = np.eye(128, dtype=bf16)
    ident2_f = (2.0 * np.eye(128)).astype(np.float32)
    ident_f8 = np.eye(128, dtype=f8)
    ones_col = np.ones((128, 1), dtype=f8)
    return ident_bf, ident2_f, ones_col, ident_f8


def build():
    nc = bacc.Bacc("TRN2", target_bir_lowering=False, debug=False,
                   num_devices=N_CORES)

    xt_d = nc.dram_tensor("xt", [128, NKT, TOK], F8, kind="ExternalInput").ap()
    wq_d = nc.dram_tensor("wq", [NCT, 128, NKT, 128], F8, kind="ExternalInput").ap()
    wd_d = nc.dram_tensor("wd", [NOC, 128, NKT, 128], F8, kind="ExternalInput").ap()
    bq_d = nc.dram_tensor("bq", [128, NCT], F32, kind="ExternalInput").ap()
    al_d = nc.dram_tensor("al", [NH, 1, TOK], BF16, kind="ExternalInput").ap()
    res_d = nc.dram_tensor("res", [NOC, 128, TOK], BF16, kind="ExternalInput").ap()
    out_d = nc.dram_tensor("out", [NOC, 128, TOK], BF16, kind="ExternalOutput").ap()

    ident_np, ident2_np, ones_np, identf8_np = _consts()
    identc_d = nc.inline_tensor(ident_np, "identc").ap()
    ident2_d = nc.inline_tensor(ident2_np, "ident2c").ap()
    onesc_d = nc.inline_tensor(ones_np, "onesc").ap()
    identf8c_d = nc.inline_tensor(identf8_np, "identf8c").ap()

    with tile.TileContext(nc) as tc, ExitStack() as ctx:
        const = ctx.enter_context(tc.tile_pool(name="const", bufs=1))
        big = ctx.enter_context(tc.tile_pool(name="big", bufs=1))
        work = ctx.enter_context(tc.tile_pool(name="work", bufs=3))
        wstream = ctx.enter_context(tc.tile_pool(name="wstream", bufs=2))
        dram = ctx.enter_context(tc.tile_pool(name="dram", bufs=1, space="DRAM"))

        # ---- persistent SBUF tensors ----
        # xt chunk 0 + the first weight tile go out FIRST so the GEMM can
        # start ~3us in instead of ~15us; consts follow.
        xt_sb = big.tile([128, NKT, TOK], F8, tag="xt")         # 16KB/part
        nc.sync.dma_start(xt_sb[:, 0:8, :], xt_d[:, 0:8, :])
        wt0 = wstream.tile([128, NKT * 128], F8, tag="w")
        nc.sync.dma_start(wt0[:], wq_d[0])
        for c4 in range(1, 4):
            nc.sync.dma_start(xt_sb[:, 8 * c4:8 * (c4 + 1), :],
                              xt_d[:, 8 * c4:8 * (c4 + 1), :])

        identity = const.tile([128, 128], BF16)
        nc.sync.dma_start(identity[:], identc_d[:])
        ident2 = const.tile([128, 128], F32)
        nc.sync.dma_start(ident2[:], ident2_d[:])
        ones_col = const.tile([128, 1], F8)
        nc.sync.dma_start(ones_col[:], onesc_d[:])
        identf8 = const.tile([128, 128], F8)
        nc.sync.dma_start(identf8[:], identf8c_d[:])
        bq_sb = const.tile([128, NCT], F32)
        nc.sync.dma_start(bq_sb[:], bq_d[:])
        # q,k token-contiguous (gram + offset need them); v only scattered.
        qk = big.tile([128, 2 * NH, TOK], BF16, tag="qk")       # 64KB/part
        # scattered fp8 copies for attention: col = c*512 + g*128 + q*32 + i
        # (i = head, c*16+g*4+q = token). q-slot-major within each 128-col
        # group makes the per-token 32x32 cross-head blocks CONTIGUOUS
        # 32-partition ranges after transposition, so the QK^T and EV
        # matmuls run as 4 concurrent 32-wide tile_position matmuls per
        # group with no cross-token waste (no block-diagonal mask needed).
        qsc = big.tile([128, NH * TOK], F8, tag="qsc")          # 16KB/part
        ksc = big.tile([128, NH * TOK], F8, tag="ksc")          # 16KB/part
        vsc = big.tile([128, NH * TOK], F8, tag="vsc")          # 16KB/part

        def scat(t, i):
            return t[:].rearrange("p (c g q i) -> p c g q i",
                                  g=4, q=4, i=NH)[:, :, :, :, i]

        # ---- GEMM1 for one qkv col-tile (fp8 DoubleRow over kt pairs) ----
        def gemm1_ct(h, off, wt=None):
            ct = 3 * h + off
            if wt is None:
                wt = wstream.tile([128, NKT * 128], F8, tag="w")
                nc.sync.dma_start(wt[:], wq_d[ct])
            ps = psA.tile([128, TOK], F32, tag="g1")
            for p in range(NKT // 2):
                lhsT = wt[:, p * 256:(p + 1) * 256].rearrange(
                    "q (two f) -> q two f", two=2)
                nc.tensor.matmul(ps[:], lhsT=lhsT,
                                 rhs=xt_sb[:, 2 * p:2 * p + 2, :],
                                 start=(p == 0), stop=(p == NKT // 2 - 1),
                                 perf_mode=DR)
            # PSUM->SBUF + 1/WSCALE + per-partition bias add on ScalarE
            bias = bq_sb[:, ct:ct + 1]
            act = mybir.ActivationFunctionType.Identity
            if off == 2:   # v: straight to the scattered fp8 buffer
                nc.scalar.activation(scat(vsc, h), ps[:], act, bias=bias,
                                     scale=1.0 / WSCALE)
            else:
                dst = qk[:, 2 * h + off, :]
                nc.scalar.activation(dst, ps[:], act, bias=bias,
                                     scale=1.0 / WSCALE)
                if off == 0:   # q: also scatter an fp8 copy for attention
                    nc.vector.tensor_copy(
                        scat(qsc, h),
                        dst.rearrange("p (c g q) -> p c g q", g=4, q=4))

        # ---- partial Gram for one head (needs qkv col-tile 3h) ----
        g_part = big.tile([128, NH * 128], BF16, tag="gx")       # 8KB/part
        g_pd = dram.tile([NH, 128, 128], BF16)
        g_sd = dram.tile([NH, 128, 128], BF16)

        qts = {}

        Msb = big.tile([128, NH * 128], BF16, tag="m")           # 8KB/part
        Gb = big.tile([128, NH * 128], BF16, tag="gb")           # 8KB/part

        def Mh(h):
            return Msb[:, h * 128:(h + 1) * 128]

        def Gh_(h):
            return Gb[:, h * 128:(h + 1) * 128]

        def g_allreduce(c4):
            nc.gpsimd.collective_compute(
                "AllReduce", mybir.AluOpType.add,
                replica_groups=[[0, 1, 2, 3], [4, 5, 6, 7]],
                ins=[g_pd[4 * c4:4 * c4 + 4].opt()],
                outs=[g_sd[4 * c4:4 * c4 + 4].opt()])

        with tc.tile_pool(name="ph1", bufs=3) as ph1, \
             tc.tile_pool(name="psA", bufs=2, space="PSUM") as psA:

            # ---------------- phase 1: q GEMM + gram ----------------
            with tc.tile_pool(name="ps1", bufs=2, space="PSUM") as ps1:
                def gram_tr(h):
                    qt = ph1.tile([128, 4 * 128], BF16, tag="qt", bufs=2)
                    pst = ps1.tile([128, 4, 128], BF16, tag="tr")
                    for t in range(4):
                        nc.tensor.transpose(pst[:, t, :],
                                            qk[:, 2 * h, t * 128:(t + 1) * 128],
                                            identity[:])
                    nc.vector.tensor_copy(
                        qt[:].rearrange("p (t f) -> p t f", t=4), pst[:])
                    qts[h] = qt

                def gram_mm(h):
                    qt = qts.pop(h)
                    pg = ps1.tile([128, 128], F32, tag="gram", bufs=1)
                    for t in range(4):
                        nc.tensor.matmul(pg[:], lhsT=qt[:, t * 128:(t + 1) * 128],
                                         rhs=qt[:, t * 128:(t + 1) * 128],
                                         start=(t == 0), stop=(t == 3))
                    nc.vector.tensor_copy(g_part[:, h * 128:(h + 1) * 128], pg[:])
                    nc.sync.dma_start(g_pd[h], g_part[:, h * 128:(h + 1) * 128])

                for h in range(NH):
                    gemm1_ct(h, 0, wt=wt0 if h == 0 else None)
                    gram_tr(h)
                    if h > 0:
                        gram_mm(h - 1)
                    if h % 4 == 0 and h >= 4:
                        g_allreduce(h // 4 - 1)
                gram_mm(NH - 1)
                g_allreduce(NH // 4 - 1)

            # ---------------- phase 2: k/v GEMM + NS + offset ----------------
            # Aux work is emitted in "slots" (2 per kv-head) so it lands
            # inside the GEMM stream with dependencies that cleared a slot
            # or more earlier (no PE stalls).
            def gb_load(c):
                # One batched DMA per chunk on the ScalarE HWDGE queue: not
                # blocked behind weight DMAs, and cheap to issue (a 32-DMA
                # burst here blocked the kv-activation stream for ~10us).
                nc.scalar.dma_start(
                    Gb[:, c * 512:(c + 1) * 512].rearrange(
                        "p (h f) -> p h f", h=4),
                    g_sd[4 * c:4 * c + 4].rearrange("h p f -> p h f"))

            def prescale(c):
                # chunk-fused: one reduce/all-reduce/mul/recip for 4 heads
                Gc = Gb[:, c * 512:(c + 1) * 512].rearrange(
                    "p (h f) -> p h f", h=4)
                rn = ph1.tile([128, 4], F32, tag="rn")
                nc.vector.tensor_reduce(rn[:], Gc,
                                        axis=mybir.AxisListType.X,
                                        op=mybir.AluOpType.add,
                                        apply_absolute_value=True)
                rmax = ph1.tile([128, 4], F32, tag="rmax")
                nc.gpsimd.partition_all_reduce(rmax[:], rn[:], 128,
                                               bass_isa.ReduceOp.max)
                rec = ph1.tile([128, 4], F32, tag="rec")
                nc.vector.tensor_mul(rec[:], rmax[:], rmax[:])
                nc.vector.reciprocal(rec[:], rec[:])
                for j in range(4):
                    h = 4 * c + j
                    nc.vector.tensor_scalar_mul(Mh(h), Gh_(h), rec[:, j:j + 1])

            t2s = {}

            def ns_p1(b, i):
                p1 = psNS.tile([128, 4, 128], F32, tag="ns1", bufs=2)
                for j in range(4):
                    h = 4 * b + j
                    nc.tensor.matmul(p1[:, j, :], lhsT=Gh_(h), rhs=Mh(h),
                                     start=True, stop=True)
                t2 = ph1.tile([128, 4, 128], BF16, tag="nst", bufs=4)
                for j in range(4):
                    nc.vector.scalar_tensor_tensor(
                        t2[:, j, :], in0=p1[:, j, :], scalar=-1.0,
                        in1=ident2[:],
                        op0=mybir.AluOpType.mult, op1=mybir.AluOpType.add)
                t2s[(b, i)] = t2

            def ns_p2(b, i):
                t2 = t2s.pop((b, i))
                p2 = psNS.tile([128, 4, 128], F32, tag="ns2", bufs=1)
                for j in range(4):
                    h = 4 * b + j
                    nc.tensor.matmul(p2[:, j, :], lhsT=Mh(h), rhs=t2[:, j, :],
                                     start=True, stop=True)
                nc.scalar.copy(Msb[:, 4 * b * 128:(4 * b + 4) * 128],
                               p2[:].rearrange("p h f -> p (h f)"))

            abs_ = {}

            def alibi_bc(h):
                # whole alibi pipeline on GpSimd (own DMA queue + broadcast);
                # keeps it off the ACT/Sync queues entirely.
                a1 = ph1.tile([1, TOK], BF16, tag="a1", bufs=4)
                nc.gpsimd.dma_start(a1[:], al_d[h])
                ab = ph1.tile([128, TOK], BF16, tag="ab", bufs=12)
                nc.gpsimd.partition_broadcast(ab[:], a1[:])
                abs_[h] = ab

            # offset_k for one head -> scattered fp8 (stored as k + bias/C1,
            # rms ~1 so fp8-safe; the C1 logit scale is applied inside the
            # Exp activation; alibi host-scaled by SCALE/C1)
            def offset_k(h):
                ab = abs_.pop(h)
                pmq = psBias.tile([128, TOK], F32, tag="mq", bufs=3)
                nc.tensor.matmul(pmq[:], lhsT=Mh(h),
                                 rhs=qk[:, 2 * h, :], start=True, stop=True)
                t1 = ph1.tile([128, TOK], BF16, tag="t1", bufs=2)
                nc.vector.tensor_mul(t1[:], pmq[:], ab[:])
                nc.vector.scalar_tensor_tensor(
                    scat(ksc, h),
                    in0=qk[:, 2 * h + 1, :].rearrange(
                        "p (c g q) -> p c g q", g=4, q=4),
                    scalar=1.0, in1=t1[:].rearrange(
                        "p (c g q) -> p c g q", g=4, q=4),
                    op0=mybir.AluOpType.mult, op1=mybir.AluOpType.add)

            # ---- slot schedule ----
            slots = {}

            def at(s, fn, *a):
                slots.setdefault(s, []).append((fn, a))

            # Gram gathers + prescales, paced behind the AllReduce chunks
            # (chunk c lands ~(95 + 18*c)us; slot s runs ~(180 + 2.1*s)us).
            gb_slot = {0: 0, 1: 0, 2: 2, 3: 2, 4: 4, 5: 6, 6: 10, 7: 14}
            for c in range(8):
                at(gb_slot[c], gb_load, c)
                at(gb_slot[c] + 1, prescale, c)
            # Two parallel NS chains (even/odd blocks), offset by one slot so
            # each slot carries one p1 OR one p2 per chain; p2 trails p1 by a
            # slot so the PE never waits on the DVE.
            ns_done = {}
            for chain in range(2):
                s = 6 + chain
                for k in range(4):
                    b = 2 * k + chain
                    for i in range(NS_ITERS):
                        at(s, ns_p1, b, i)
                        at(s + 1, ns_p2, b, i)
                        s += 2
                    ns_done[b] = s
            # offset_k(h) reads the k row written by gemm1_ct(h, 1), which is
            # emitted at slot 2h; it MUST be emitted at slot >= 2h+2 (else it
            # reads uninitialized qk). Also needs its NS block + alibi bcast.
            # First 12 broadcasts fit in the ab buffers outright and may run
            # any time; later ones go after the prescale PARs (slot >= 17) so
            # a broadcast blocked on an ab slot never heads-of-line-blocks a
            # PAR in the GpSimd queue.
            for h in range(NH):
                s_off = max(2 * h + 1, ns_done[h // 4] + 1 + (h % 4))
                at(2 if h < 12 else max(17, s_off - 4), alibi_bc, h)
                at(s_off, offset_k, h)

            # Modeled-time floors: the scheduler's matmul model (~107ns/DR-MM)
            # is ~2.5x faster than measured (~263ns), while its collective
            # model is ~1.8x slower than measured. Without correction it
            # believes the GEMM finishes before the AllReduce chain and
            # pushes all NS/offset work into a serial tail. Flooring each
            # col-tile at its real pace stretches the modeled GEMM so the
            # aux chain interleaves into the stream. Floors only affect
            # scheduling; they emit no runtime waits.
            with tc.tile_pool(name="psNS", bufs=2, space="PSUM") as psNS, \
                 tc.tile_pool(name="psBias", bufs=2, space="PSUM") as psBias:
                for h in range(NH):
                    for fn, a in slots.pop(2 * h, []):
                        fn(*a)
                    with tc.tile_wait_until(0.205 + h * 0.0078):
                        gemm1_ct(h, 1)
                    for fn, a in slots.pop(2 * h + 1, []):
                        fn(*a)
                    with tc.tile_wait_until(0.209 + h * 0.0078):
                        gemm1_ct(h, 2)
                for s in sorted(slots):
                    for fn, a in slots[s]:
                        fn(*a)

        # ---- attention: 32 chunks of 16 positions (4 groups of 4) ----
        # Transposed-context formulation: ctxT[(i,q), d] = ef^T @ v^T with a
        # ones column appended to v^T so the softmax denominator falls out of
        # the same matmul; normalization is then a per-PARTITION
        # tensor_scalar_mul (no GpSimd broadcast, no reciprocal transposes,
        # no colsum matmuls), and a PE transpose brings ctx back to [hd, tok].
        # All operands are contiguous fp8 slices of the scattered buffers.
        # Two-stage software pipeline so ctxT mms never stall the PE queue.
        ctx3 = big.tile([128, NH, TOK], F8, tag="xt")            # reuse xt slot
        vps = [big.tile([128, 4, 132], F8, tag=f"vps{i}", name=f"vps{i}")
               for i in range(2)]
        for i in range(2):
            for g in range(4):
                nc.vector.tensor_copy(vps[i][:, g, 128:129], ones_col[:])
        with tc.tile_pool(name="psB", bufs=2, space="PSUM") as psB, \
             tc.tile_pool(name="awork", bufs=3) as awork:
            NCH = NGRP // 4
            efs = {}
            ctss = {}

            def att_a(ch):
                c0 = 512 * ch
                # fp8 transpose mode requires output element step of 2
                pv = psB.tile([128, 4, 256], F8, tag="pv", bufs=1)
                pw = psB.tile([128, 4, 32], F32, tag="w", bufs=2)
                for g in range(4):
                    sl = slice(c0 + 128 * g, c0 + 128 * (g + 1))
                    nc.tensor.transpose(pv[:, g, 0:256:2], vsc[:, sl],
                                        identf8[:])
                # per-token 32x32 cross-head logits only: 4 concurrent
                # col-group matmuls per group, no cross-token waste
                for g in range(4):
                    base = c0 + 128 * g
                    for qq in range(4):
                        nc.tensor.matmul(
                            pw[32 * qq:32 * qq + 32, g, :],
                            lhsT=ksc[:, base + 32 * qq:base + 32 * qq + 32],
                            rhs=qsc[:, base + 32 * qq:base + 32 * qq + 32],
                            start=True, stop=True,
                            tile_position=(0, 32 * qq))
                nc.scalar.copy(vps[ch % 2][:, :, 0:128], pv[:, :, 0:256:2])
                ef = awork.tile([128, 4, 32], F8, tag="ef")
                nc.scalar.activation(ef[:].rearrange("p g f -> p (g f)"),
                                     pw[:].rearrange("p g f -> p (g f)"),
                                     mybir.ActivationFunctionType.Exp, scale=C1)
                efs[ch] = ef

            def att_b1(ch):
                ef = efs.pop(ch)
                # [128, 2, 256] so each group's 129-col slice is bank-aligned
                # even with a 32-partition tile_position base offset
                pcT0 = psB.tile([128, 2, 256], F32, tag="pcT0", bufs=2)
                pcT1 = psB.tile([128, 2, 256], F32, tag="pcT1", bufs=2)
                pcT = lambda g: (pcT0 if g < 2 else pcT1)[:, g % 2, :]
                # diagonal 32x32 tiles: contraction restricted to each
                # token's own kv heads, denominator from the ones column
                for g in range(4):
                    pct_t = pcT0 if g < 2 else pcT1
                    for qq in range(4):
                        nc.tensor.matmul(
                            pct_t[32 * qq:32 * qq + 32, g % 2, 0:129],
                            lhsT=ef[32 * qq:32 * qq + 32, g, :],
                            rhs=vps[ch % 2][32 * qq:32 * qq + 32, g, 0:129],
                            start=True, stop=True,
                            tile_position=(32 * qq, 32 * qq))
                rr = awork.tile([128, 4], F32, tag="rr")
                nc.vector.tensor_scalar_add(
                    rr[:, 0:2], pcT0[:, :, 128:129].rearrange("p a b -> p (a b)"),
                    EPS)
                nc.vector.tensor_scalar_add(
                    rr[:, 2:4], pcT1[:, :, 128:129].rearrange("p a b -> p (a b)"),
                    EPS)
                nc.vector.reciprocal(rr[:], rr[:])
                cts = awork.tile([128, 4, 128], F8, tag="cts")
                nc.scalar.activation(cts[:, 0, :], pcT(0)[0:128, 0:128],
                                     mybir.ActivationFunctionType.Identity,
                                     scale=rr[:, 0:1])
                for g in range(1, 4):
                    nc.vector.tensor_scalar_mul(cts[:, g, :],
                                                pcT(g)[0:128, 0:128],
                                                rr[:, g:g + 1])
                ctss[ch] = cts

            def att_b2(ch):
                c0 = 16 * ch
                cts = ctss.pop(ch)
                pctx = psB.tile([128, 4, 256], F8, tag="pctx", bufs=1)
                for g in range(4):
                    nc.tensor.transpose(pctx[:, g, 0:256:2], cts[:, g, :],
                                        identf8[:])
                dst = ctx3[:, :, c0:c0 + 16].rearrange("p i (g q) -> p g i q", q=4)
                src = pctx[:, :, 0:256:2].rearrange("p g (q i) -> p g i q", q=4)
                nc.scalar.copy(dst, src)

            for ch in range(NCH):
                att_a(ch)
                if ch >= 1:
                    att_b1(ch - 1)
                if ch >= 2:
                    att_b2(ch - 2)
            att_b1(NCH - 1)
            att_b2(NCH - 2)
            att_b2(NCH - 1)

        # ---- dense + residual (fp8 DoubleRow over head pairs) ----
        with tc.tile_pool(name="psC", bufs=2, space="PSUM") as psC:
            for oc in range(NOC):
                wt = wstream.tile([128, NKT * 128], F8, tag="w")
                nc.sync.dma_start(wt[:], wd_d[oc])
                ps = psC.tile([128, TOK], F32, tag="dn")
                for p in range(NKT // 2):
                    lhsT = wt[:, p * 256:(p + 1) * 256].rearrange(
                        "q (two f) -> q two f", two=2)
                    nc.tensor.matmul(ps[:], lhsT=lhsT,
                                     rhs=ctx3[:, 2 * p:2 * p + 2, :],
                                     start=(p == 0), stop=(p == NKT // 2 - 1),
                                     perf_mode=DR)
                rs = work.tile([128, TOK], BF16, tag="rs", bufs=4)
                nc.sync.dma_start(rs[:], res_d[oc])
                ot = work.tile([128, TOK], BF16, tag="ot", bufs=3)
                nc.vector.scalar_tensor_tensor(
                    ot[:], in0=ps[:], scalar=1.0 / WSCALE, in1=rs[:],
                    op0=mybir.AluOpType.mult, op1=mybir.AluOpType.add)
                nc.sync.dma_start(out_d[oc], ot[:])

    nc.compile()
    return nc


def _prep_host(hidden_states, residual, alibi, W_qkv, b_qkv, W_dense, b_dense):
    wq_host = np.ascontiguousarray(
        np.clip(W_qkv.T * WSCALE, -240, 240).astype(f8)
        .reshape(NKT, 128, NCT, 128).transpose(2, 1, 0, 3))
    wd_host = np.ascontiguousarray(
        np.clip(W_dense.T * WSCALE, -240, 240).astype(f8)
        .reshape(NKT, 128, NOC, 128).transpose(2, 1, 0, 3))
    bq_host = np.ascontiguousarray(
        b_qkv.astype(np.float32).reshape(NCT, 128).T)
    al3 = alibi.reshape(B, NH, S)
    in_maps = []
    for c in range(N_CORES):
        b, t0 = c // 4, (c % 4) * TOK
        xt = np.ascontiguousarray(
            np.clip(hidden_states[b, t0:t0 + TOK, :].T, -240, 240).astype(f8)
            .reshape(NKT, 128, TOK).transpose(1, 0, 2))
        al = np.ascontiguousarray(
            (al3[b, :, t0:t0 + TOK] * (SCALE / C1)).astype(bf16)[:, None, :])
        res = np.ascontiguousarray(
            (residual[b, t0:t0 + TOK, :].T + b_dense[:, None])
            .astype(bf16).reshape(NOC, 128, TOK))
        in_maps.append({"xt": xt, "wq": wq_host, "wd": wd_host,
                        "bq": bq_host, "al": al, "res": res})
    return in_maps


def kernel(hidden_states, residual, alibi, attention_mask,
           W_qkv, b_qkv, W_dense, b_dense):
    hidden_states = np.asarray(hidden_states, dtype=np.float32)
    residual = np.asarray(residual, dtype=np.float32)
    alibi = np.asarray(alibi, dtype=np.float32)
    W_qkv = np.asarray(W_qkv, dtype=np.float32)
    b_qkv = np.asarray(b_qkv, dtype=np.float32)
    W_dense = np.asarray(W_dense, dtype=np.float32)
    b_dense = np.asarray(b_dense, dtype=np.float32)

    if "nc" not in _CACHE:
        _CACHE["nc"] = build()
    nc = _CACHE["nc"]

    in_maps = _prep_host(hidden_states, residual, alibi, W_qkv, b_qkv,
                         W_dense, b_dense)
    res = run_bass_kernel_spmd(nc, in_maps, list(range(N_CORES)))
    _CACHE["last_result"] = res

    out = np.empty((B, S, H), dtype=np.float32)
    for c in range(N_CORES):
        b, t0 = c // 4, (c % 4) * TOK
        ot = res.results[c]["out"]              # [NOC,128,TOK] bf16
        out[b, t0:t0 + TOK, :] = ot.reshape(H, TOK).T.astype(np.float32)
    return out
